# revision 8
# baseline (speedup 1.0000x reference)
"""KWTA (k-winners-take-all) Trainium2 kernel.

Reference semantics (B=32768, D=2048, K=40, ALPHA=0.01, GAMMA=1.0):
    _, idx = top_k(x, K); mask = one_hot_k(idx)           # [B, D]
    new_duty = duty*(1-ALPHA) + ALPHA*mean(mask, axis=0)  # [1, D]
    boost = exp(-GAMMA*(new_duty - K/D))                  # [1, D]
    out = x * boost * mask

Sharding: batch dim across 8 cores (4096 rows each). Two SPMD launches:
  K1: per 128-row tile, 5 rounds of (DVE max8 -> match_replace sentinel)
      destroys a copy of x in SBUF; winners become -1e30. Mask = sentinel
      compare (exact top-k selection incl. value ties, matching
      jax.lax.top_k's lowest-index-first tie rule). Mask (bf16) -> DRAM,
      per-column counts via PE matmul(ones^T @ mask) -> DRAM.
  Host: sum counts over cores (exact f32 ints), EMA + exp -> boost [1, D].
  K2: out = (x .* bcast(boost)) .* mask.
"""

import numpy as np

import concourse.bass as bass
import concourse.mybir as mybir
import concourse.tile as tile
from concourse.tile import ScopedClock
from concourse.bass_utils import run_bass_kernel_spmd

B, D, K = 32768, 2048, 40
N_CORES = 8
ROWS = B // N_CORES          # 4096 rows per core
P = 128                      # partitions
NT = ROWS // P               # 32 tiles per core
ALPHA = 0.01
TARGET = K / D
SENT = -1.0e30               # match_replace sentinel
F32 = mybir.dt.float32
BF16 = mybir.dt.bfloat16


def _patch_drain():
    """This container's walrus caps sync-waits per CTRL instruction below what
    Tile's tail drain emits. Split the drain's vector-clock waits across
    one nop per logical proc; the drain itself then needs no waits (same-engine
    program order)."""
    if getattr(tile.TileContext, "_drain_split_patched", False):
        return

    def patched(self, tick_clock, wait_clock):
        nc = self.nc
        gc = tick_clock.global_clock
        VC = type(gc)
        NPROCS = 27
        for p in range(NPROCS):
            try:
                v = gc[p]
            except Exception:
                v = 0
            if v <= 0:
                continue
            partial = [0] * NPROCS
            partial[p] = v
            nop = nc.sync.nop(nofuse=True, hint=f"drain_split_{p}")
            wait_clock.add_sem_waits(nop.ins, ScopedClock({None: VC(partial)}))
        nc.sync.drain()
        nc.all_engine_barrier()
        assert self.sems is not None
        popped = nc._tile_sem_poison_stack.pop()
        assert popped is self._sem_poison
        nc.clear_and_free_semaphores(list(self.sems.allocated().values()))
        nc.all_engine_barrier()

    tile.TileContext._drain_and_barrier = patched
    tile.TileContext._drain_split_patched = True


_patch_drain()


def _split_waits_json(bir_json):
    """This walrus build rejects >1 sem-wait per instruction. Rewrite the BIR:
    hoist all but the last wait of each instruction onto NoOps injected just
    before it on the same engine stream (sound: nothing intervenes on that
    engine, and a DMA descriptor cannot execute before it is enqueued)."""
    import json as _json
    if isinstance(bir_json, bytes):
        j = _json.loads(bir_json.decode())
    else:
        j = _json.loads(bir_json)
    n = 0
    for fn in j.get("functions", []):
        for blk in fn.get("blocks", []):
            insts = blk.get("instructions", [])
            if not any(
                len(((ins.get("sync_info") or {}).get("on_wait") or [])) > 1
                for ins in insts
            ):
                continue
            out = []
            for ins in insts:
                si = ins.get("sync_info") or {}
                ow = si.get("on_wait") or []
                if len(ow) > 1:
                    for w in ow[:-1]:
                        out.append({
                            "debug": ins.get("debug", 0),
                            "engine": ins["engine"],
                            "ins": [],
                            "outs": [],
                            "name": f"WSPLIT-{n}",
                            "opcode": "NoOp",
                            "sync_info": {"on_update": [], "on_wait": [w]},
                            "text_hint": "wait_split",
                        })
                        n += 1
                    si["on_wait"] = [ow[-1]]
                out.append(ins)
            blk["instructions"] = out
    return _json.dumps(j).encode()


def _patch_compile():
    import concourse.bass_utils as bu
    if getattr(bu, "_wsplit_patched", False):
        return
    orig = bu._compile_bir_impl

    def wrapped(bir_json, *a, **k):
        return orig(_split_waits_json(bir_json), *a, **k)

    bu._compile_bir_impl = wrapped
    bu._wsplit_patched = True


_patch_compile()


def k1_body(tc, x_ap, mask_ap, counts_ap, nt):
    """Top-k mask + per-column counts for nt 128-row tiles."""
    nc = tc.nc
    xt = x_ap.rearrange("(n p) d -> n p d", p=P)
    mt = mask_ap.rearrange("(n p) d -> n p d", p=P)
    with (
        tc.tile_pool(name="work", bufs=4) as pool,
        tc.tile_pool(name="cst", bufs=1) as cpool,
        tc.tile_pool(name="acc", bufs=1, space="PSUM") as ppool,
    ):
        ones = cpool.tile([P, 1], BF16, tag="ones")
        nc.vector.memset(ones[:], 1.0)
        nbias = cpool.tile([P, 1], F32, tag="nbias")
        nc.vector.memset(nbias[:], -1.0e29)
        cnt_ps = [
            ppool.tile([1, 512], F32, tag=f"cnt{j}", name=f"cnt{j}")
            for j in range(4)
        ]

        for i in range(nt):
            tmp = pool.tile([P, D], F32, tag="tmp")
            nc.sync.dma_start(tmp[:], xt[i])
            m8 = pool.tile([P, 8], F32, tag="m8")
            for _ in range(K // 8):
                nc.vector.max(out=m8[:], in_=tmp[:])
                nc.vector.match_replace(
                    out=tmp[:], in_to_replace=m8[:], in_values=tmp[:],
                    imm_value=SENT,
                )
            # winners are SENT; mask = 1 where tmp <= -1e29 (ACT engine, DVE stays free)
            sgn = pool.tile([P, D], F32, tag="sgn")
            nc.scalar.activation(
                sgn[:], tmp[:], mybir.ActivationFunctionType.Sign,
                bias=nbias[:], scale=-1.0,
            )  # winner -> +1, other -> -1
            mask = pool.tile([P, D], BF16, tag="mask")
            nc.scalar.activation(
                mask[:], sgn[:], mybir.ActivationFunctionType.Copy,
                bias=0.5, scale=0.5,
            )  # -> {0, 1}
            for j in range(4):
                nc.tensor.matmul(
                    cnt_ps[j][:], lhsT=ones[:], rhs=mask[:, j * 512:(j + 1) * 512],
                    start=(i == 0), stop=(i == nt - 1),
                )
            nc.sync.dma_start(mt[i], mask[:])

        csb = pool.tile([1, D], F32, tag="csb")
        for j in range(4):
            nc.scalar.copy(csb[0:1, j * 512:(j + 1) * 512], cnt_ps[j][0:1, :])
        nc.sync.dma_start(counts_ap[:], csb[:])


def k2_body(tc, x_ap, mask_ap, boost_ap, out_ap, nt):
    """out = x * bcast(boost) * mask."""
    nc = tc.nc
    xt = x_ap.rearrange("(n p) d -> n p d", p=P)
    mt = mask_ap.rearrange("(n p) d -> n p d", p=P)
    ot = out_ap.rearrange("(n p) d -> n p d", p=P)
    with (
        tc.tile_pool(name="work", bufs=4) as pool,
        tc.tile_pool(name="cst", bufs=1) as cpool,
        tc.tile_pool(name="bps", bufs=1, space="PSUM") as ppool,
    ):
        # broadcast boost [1, D] -> [P, D] via PE (ones[1,P]^T @ boost)
        b1 = cpool.tile([1, D], F32, tag="b1")
        nc.sync.dma_start(b1[:], boost_ap[:])
        onesf = cpool.tile([1, P], F32, tag="onesf")
        nc.vector.memset(onesf[:], 1.0)
        bb = cpool.tile([P, D], F32, tag="bb")
        for j in range(4):
            bps = ppool.tile([P, 512], F32, tag=f"b{j}")
            nc.tensor.matmul(
                bps[:], lhsT=onesf[:], rhs=b1[0:1, j * 512:(j + 1) * 512],
                start=True, stop=True,
            )
            nc.scalar.copy(bb[:, j * 512:(j + 1) * 512], bps[:])

        for i in range(nt):
            xt_t = pool.tile([P, D], F32, tag="xt")
            nc.sync.dma_start(xt_t[:], xt[i])
            mk = pool.tile([P, D], BF16, tag="mk")
            nc.sync.dma_start(mk[:], mt[i])
            t1 = pool.tile([P, D], F32, tag="t1")
            nc.vector.tensor_tensor(
                out=t1[:], in0=xt_t[:], in1=bb[:], op=mybir.AluOpType.mult)
            ot_t = pool.tile([P, D], F32, tag="ot")
            nc.vector.tensor_tensor(
                out=ot_t[:], in0=t1[:], in1=mk[:], op=mybir.AluOpType.mult)
            nc.sync.dma_start(ot[i], ot_t[:])


def build_k1(rows=ROWS):
    nc = bass.Bass(num_devices=N_CORES)
    x = nc.dram_tensor("x", [rows, D], F32, kind="ExternalInput")
    mask = nc.dram_tensor("mask", [rows, D], BF16, kind="ExternalOutput")
    counts = nc.dram_tensor("counts", [1, D], F32, kind="ExternalOutput")
    with tile.TileContext(nc) as tc:
        k1_body(tc, x[:], mask[:], counts[:], rows // P)
    return nc


def build_k2(rows=ROWS):
    nc = bass.Bass(num_devices=N_CORES)
    x = nc.dram_tensor("x", [rows, D], F32, kind="ExternalInput")
    mask = nc.dram_tensor("mask", [rows, D], BF16, kind="ExternalInput")
    boost = nc.dram_tensor("boost", [1, D], F32, kind="ExternalInput")
    out = nc.dram_tensor("out", [rows, D], F32, kind="ExternalOutput")
    with tile.TileContext(nc) as tc:
        k2_body(tc, x[:], mask[:], boost[:], out[:], rows // P)
    return nc


_nc_cache = {}


def _get_nc(name, builder):
    if name not in _nc_cache:
        _nc_cache[name] = builder()
    return _nc_cache[name]


def host_boost(counts_total, duty):
    """EMA + boost, mirroring the reference's f32 ops exactly."""
    counts_total = counts_total.astype(np.float32)
    mean = counts_total / np.float32(B)
    new_duty = duty.astype(np.float32) * np.float32(1.0 - ALPHA) \
        + np.float32(ALPHA) * mean
    z = new_duty - np.float32(TARGET)
    return np.exp(-z).astype(np.float32)


LAST_HW_NS = None
LAST_TRACE_DIRS = []


def kernel(x, duty):
    global LAST_HW_NS, LAST_TRACE_DIRS
    import os
    trace = bool(int(os.environ.get("KWTA_TRACE", "0")))
    try:
        from antenv.axon_hooks import get_axon_ntff_profile_hook  # noqa: F401
    except Exception:
        trace = False
    tkw = {}
    if trace:
        import tempfile
        tkw = dict(trace=True, tmpdir=tempfile.mkdtemp(prefix="kwta_k1_"))
    x = np.ascontiguousarray(x, dtype=np.float32)
    duty = np.asarray(duty, dtype=np.float32).reshape(1, D)
    xs = x.reshape(N_CORES, ROWS, D)

    nc1 = _get_nc("k1", build_k1)
    r1 = run_bass_kernel_spmd(
        nc1, [{"x": xs[i]} for i in range(N_CORES)],
        core_ids=list(range(N_CORES)), **tkw,
    )
    counts_total = np.zeros((1, D), dtype=np.float32)
    for r in r1.results:
        counts_total += r["counts"]          # exact: integer-valued f32
    boost = host_boost(counts_total, duty)

    nc2 = _get_nc("k2", build_k2)
    in2 = [
        {"x": xs[i], "mask": r1.results[i]["mask"], "boost": boost}
        for i in range(N_CORES)
    ]
    tkw2 = {}
    if trace:
        import tempfile
        tkw2 = dict(trace=True, tmpdir=tempfile.mkdtemp(prefix="kwta_k2_"))
    r2 = run_bass_kernel_spmd(nc2, in2, core_ids=list(range(N_CORES)), **tkw2)

    if trace:
        ns = 0
        ok = True
        for r, kw in ((r1, tkw), (r2, tkw2)):
            if r.exec_time_ns is None:
                ok = False
            else:
                ns += r.exec_time_ns
        LAST_HW_NS = ns if ok else None
        LAST_TRACE_DIRS = [tkw.get("tmpdir"), tkw2.get("tmpdir")]
    return np.concatenate([r["out"] for r in r2.results], axis=0)
